# revision 7
# baseline (speedup 1.0000x reference)
"""BlockwiseDense Trainium2 kernel (8 NeuronCores, sharded over out_blocks).

Math (per reference):
    w = rram_quantize(relu(cores))          # snap to 256 log-spaced levels
    y[b,i,j,k] = sum_l w[i,j,k,l] * x[b,j,l]

Quantization is analytic.  With d = (g_min - w)/B, the continuous level
index is s(w) = MULT*ln(1+d) + C0; n = sat_u8(rne(s)); q = A - B*r^n.
Two interchangeable n-paths, balanced across engines per j-pair:
  "ln"  : t = Ln((A-w)/B)  (ACT)   ; n = sat_u8(t*MULT + C0)    (DVE ts)
  "poly": s ~= 170*d^2 - 340*d + C0 = (w + c1')*(alpha2*w + beta2)
          one ts (a2c) + one scalar_tensor_tensor -> sat_u8      (DVE/GpSimd)
then e = Exp(n*ln_r) (ACT, fp32) ; q = f16(A - B*e) (GpSimd ts);
fp16 matmuls with 512-wide rhs (both i-blocks fused into the moving
operand), fp32 PSUM accumulation over the two 128-row halves of l,
DVE cast-evict to fp16, store.

Host pre-casts weights to fp16 (halves the dominant DMA stream;
validated ~5e-3 rel err) with layouts:
    xt: (p, j, h, b)         x^T fp16, 8KB contiguous per partition
    wt: (jp, p, jj, h, i, k) cores^T fp16, 4KB contiguous per partition
Engine streams are software-pipelined so ACT never waits on the DVE
round trip.  Sharding: core c takes out_blocks {2c, 2c+1}.
"""

import numpy as np

import concourse.bacc as bacc
import concourse.mybir as mybir
from concourse.tile import TileContext
from concourse.bass_utils import run_bass_kernel_spmd

# ---- problem constants (hardcoded per contract) ----
BATCH = 128
IN_BLOCKS = 16
OUT_BLOCKS = 16
NB = 256
N_CORES = 8
I_PER_CORE = OUT_BLOCKS // N_CORES  # 2
JP = IN_BLOCKS // 2  # 8 j-pairs

TAU, G_INF, G_MIN, L = 0.75, 2.0, 0.001, 256
B_SCALE = (G_INF - G_MIN) / (1.0 - float(np.exp(-TAU)))
A_OFF = G_MIN + B_SCALE
MULT = -(L - 1) / TAU
LN_R = -TAU / (L - 1)
_r = float(np.exp(LN_R))
DELTA = float(np.log((1 + _r) / 2) / LN_R)
C0 = 0.5 - DELTA  # rne(s) == floor(s+0.5): n = floor(t + 1 - delta)

# poly-n path: s(w) ~= (d + c1)*(170*d + g2) with d = (g_min - w)/B,
# g2 = -340 - 170*c1, c1 = -C0/340 (so the constant term lands on C0).
# In w-coordinates: s = (w + C1W) * (A2M*w + A2B)
_c1 = -C0 / 340.0
_g2 = -340.0 - 170.0 * _c1
C1W = -G_MIN + B_SCALE * _c1          # w + C1W = -B*(d + c1) ... sign folded:
# d + c1 = (g_min - w)/B + c1 = -(w - g_min - B*c1)/B
# 170*d + g2 = -(170/B)*w + 170*g_min/B + g2
# s = (d+c1)(170 d + g2) = (w - (g_min + B*c1)) * ((170/B**2)*w - (170*g_min/B + g2)/B)
C1W = -(G_MIN + B_SCALE * _c1)        # scalar added to w in STT
A2M = 170.0 / (B_SCALE * B_SCALE)
A2B = -(170.0 * G_MIN / B_SCALE + _g2) / B_SCALE

F32 = mybir.dt.float32
F16 = mybir.dt.float16
U8 = mybir.dt.uint8

FD = 2 * 2 * I_PER_CORE * NB  # 2048 free elems per partition per j-pair
IK = I_PER_CORE * NB  # 512

# ---- per-granule tuning knobs ----
PATH = ["ln", "ln", "poly", "poly", "poly", "poly", "poly", "poly"]
A2_ENG = ["v", "v", "g", "g", "g", "v", "v", "v"]  # a2c ts engine (poly)
Q_ENG = ["g", "g", "g", "g", "g", "g", "g", "g"]  # q-affine engine

_CACHE = {}


class _ForceCombinedLnExpTables:
    """Resolve Ln and Exp to the single table set containing both, so the
    ACT never reloads tables mid-kernel."""

    def __enter__(self):
        self._orig = bacc.get_activation_tables
        Ln = mybir.ActivationFunctionType.Ln
        Exp = mybir.ActivationFunctionType.Exp

        def patched(arch):
            tabs = self._orig(arch)
            out = {}
            for name, fns in tabs.items():
                if name != "natural_log_exp_and_others" and (Ln in fns or Exp in fns):
                    fns = fns - {Ln, Exp}
                out[name] = fns
            return out

        bacc.get_activation_tables = patched
        return self

    def __exit__(self, *exc):
        bacc.get_activation_tables = self._orig


def _build():
    nc = bacc.Bacc(trn_type="TRN2")
    P = 128

    xt_d = nc.dram_tensor("xt", [P, IN_BLOCKS, 2, BATCH], F16, kind="ExternalInput")
    wt_d = nc.dram_tensor("wt", [JP, P, 2, 2, IK], F16, kind="ExternalInput")
    y_d = nc.dram_tensor("y", [BATCH, JP, 2, IK], F16, kind="ExternalOutput")

    flat = "p a b k -> p (a b k)"
    eng = {"v": nc.vector, "g": nc.gpsimd}

    with TileContext(nc) as tc:
        with (
            tc.tile_pool(name="singles", bufs=1) as singles,
            tc.tile_pool(name="wraw", bufs=4) as wpool,
            tc.tile_pool(name="tmid", bufs=3) as tpool,
            tc.tile_pool(name="nidx", bufs=4) as npool,
            tc.tile_pool(name="texp", bufs=3) as epool,
            tc.tile_pool(name="qw", bufs=3) as qpool,
            tc.tile_pool(name="yout", bufs=3) as ypool,
            tc.tile_pool(name="yps", bufs=3, space="PSUM") as yps,
        ):
            wtiles = [None] * JP
            ttiles = [None] * JP
            ntiles = [None] * JP
            etiles = [None] * JP
            qtiles = [None] * JP
            ytiles = [None] * JP
            ptiles = [None] * JP

            def dma_w(jp):
                wtiles[jp] = wpool.tile([P, 2, 2, IK], F16, name="wraw", tag="wraw")
                nc.sync.dma_start(out=wtiles[jp][:], in_=wt_d[jp])

            # --- n-index stages ---
            def ln_stage(jp):
                ttiles[jp] = tpool.tile([P, FD], F32, name="tln", tag="tmid")
                nc.scalar.activation(
                    ttiles[jp][:],
                    wtiles[jp][:].rearrange(flat),
                    mybir.ActivationFunctionType.Ln,
                    bias=bias_ln[:, 0:1],
                    scale=-1.0 / B_SCALE,
                )

            def n_from_ln(jp):
                ntiles[jp] = npool.tile([P, FD], U8, name="nidx", tag="nidx")
                nc.vector.tensor_scalar(
                    ntiles[jp][:],
                    ttiles[jp][:],
                    MULT,
                    C0,
                    mybir.AluOpType.mult,
                    mybir.AluOpType.add,
                )

            def a2_stage(jp):
                ttiles[jp] = tpool.tile([P, FD], F16, name="a2c", tag="tmid")
                eng[A2_ENG[jp]].tensor_scalar(
                    ttiles[jp][:],
                    wtiles[jp][:].rearrange(flat),
                    A2M,
                    A2B,
                    mybir.AluOpType.mult,
                    mybir.AluOpType.add,
                )

            def n_from_poly(jp):
                ntiles[jp] = npool.tile([P, FD], U8, name="nidx", tag="nidx")
                nc.vector.scalar_tensor_tensor(
                    ntiles[jp][:],
                    wtiles[jp][:].rearrange(flat),
                    C1W,
                    ttiles[jp][:],
                    mybir.AluOpType.add,
                    mybir.AluOpType.mult,
                )

            def n_stage(jp):
                if PATH[jp] == "ln":
                    ln_stage(jp)
                else:
                    a2_stage(jp)

            def n_stage2(jp):
                if PATH[jp] == "ln":
                    n_from_ln(jp)
                else:
                    n_from_poly(jp)

            def exp_stage(jp):
                etiles[jp] = epool.tile([P, FD], F32, name="texp", tag="texp")
                nc.scalar.activation(
                    etiles[jp][:],
                    ntiles[jp][:],
                    mybir.ActivationFunctionType.Exp,
                    bias=0.0,
                    scale=LN_R,
                )

            def q_stage(jp):
                qtiles[jp] = qpool.tile([P, 2, 2, IK], F16, name="qw", tag="qw")
                eng[Q_ENG[jp]].tensor_scalar(
                    qtiles[jp][:].rearrange(flat),
                    etiles[jp][:],
                    -B_SCALE,
                    A_OFF,
                    mybir.AluOpType.mult,
                    mybir.AluOpType.add,
                )

            def mm(jp):
                ptiles[jp] = yps.tile([P, 2, IK], F32, name="yp", tag="yp")
                for jx in range(2):
                    j = 2 * jp + jx
                    for h in range(2):
                        nc.tensor.matmul(
                            ptiles[jp][:, jx],
                            xt_sb[:, j, h, :],
                            qtiles[jp][:, jx, h, :],
                            start=(h == 0),
                            stop=(h == 1),
                        )

            def evict(jp):
                ytiles[jp] = ypool.tile([P, 2, IK], F16, name="ysb", tag="ysb")
                nc.vector.tensor_copy(
                    ytiles[jp][:].rearrange("p a k -> p (a k)"),
                    ptiles[jp][:].rearrange("p a k -> p (a k)"),
                )

            def store(jp):
                nc.sync.dma_start(out=y_d[:, jp], in_=ytiles[jp][:])

            # --- prologue: weights first (they gate compute), then x ---
            dma_w(0)
            dma_w(1)
            xt_sb = singles.tile([P, IN_BLOCKS, 2, BATCH], F16)
            nc.sync.dma_start(out=xt_sb[:], in_=xt_d[:])
            bias_ln = singles.tile([P, 1], F32)
            nc.vector.memset(bias_ln[:], A_OFF / B_SCALE)

            # --- software-pipelined main loop ---
            n_stage(0)
            for jp in range(JP):
                n_stage2(jp)
                if jp + 2 < JP:
                    dma_w(jp + 2)
                if jp + 1 < JP:
                    n_stage(jp + 1)
                exp_stage(jp)
                q_stage(jp)
                mm(jp)
                evict(jp)
                store(jp)

    with _ForceCombinedLnExpTables():
        nc.compile()
    return nc


def _get_nc():
    if "nc" not in _CACHE:
        _CACHE["nc"] = _build()
    return _CACHE["nc"]


def kernel(x: np.ndarray, cores: np.ndarray, _trace=False, _trace_kwargs=None):
    x = np.asarray(x, dtype=np.float32)
    cores = np.asarray(cores, dtype=np.float32)

    xt = np.ascontiguousarray(
        x.T.reshape(IN_BLOCKS, 2, 128, BATCH)
        .transpose(2, 0, 1, 3)
        .astype(np.float16)
    )
    wt_full = (
        cores.reshape(OUT_BLOCKS, JP, 2, NB, 2, 128)  # i, jp, jj, k, h, p
        .transpose(1, 5, 2, 4, 0, 3)  # jp, p, jj, h, i, k
        .astype(np.float16)
    )

    in_maps = []
    for c in range(N_CORES):
        wc = np.ascontiguousarray(
            wt_full[:, :, :, :, c * I_PER_CORE : (c + 1) * I_PER_CORE, :]
        ).reshape(JP, 128, 2, 2, IK)
        in_maps.append({"xt": xt, "wt": wc})

    nc = _get_nc()
    kw = {}
    if _trace:
        kw = dict(trace=True, **(_trace_kwargs or {}))
    out = run_bass_kernel_spmd(nc, in_maps, core_ids=list(range(N_CORES)), **kw)
    if _trace:
        _CACHE["last_result"] = out
    y = np.concatenate(
        [
            r["y"]  # (b, jp, jj, (i,k))
            .astype(np.float32)
            .reshape(BATCH, IN_BLOCKS, I_PER_CORE, NB)
            .transpose(0, 2, 1, 3)
            for r in out.results
        ],
        axis=1,
    )
    return y


# revision 11
# speedup vs baseline: 1.3357x; 1.3357x over previous
"""BlockwiseDense Trainium2 kernel (8 NeuronCores, sharded over out_blocks).

Math (per reference):
    w = rram_quantize(relu(cores))          # snap to 256 log-spaced levels
    y[b,i,j,k] = sum_l w[i,j,k,l] * x[b,j,l]

Level index n = sat_u8(rne(s(w))), s = MULT*ln((A-w)/B) + C0.  Two
engine-balanced n-paths per granule of j-blocks:
  "ln"  : t = Ln((A-w)/B)  (ACT)  ; n = sat_u8(t*MULT + C0)  (GpSimd ts)
  "poly": s ~= w*(A2M*w + P1) + P0   [quadratic; validated 0.06% flips]
          gg = ts(w*A2M + P1)  ; hh = tt(w*gg) ; n = ts(hh + P0 -> u8)  (DVE)
Then e = Exp(n*ln_r) in fp16 (ACT) feeds the matmuls directly; the
affine q = A - B*e is folded into the PSUM eviction:
    y = (psum + SA[b,j]) * (-B),   SA = -(A/B) * S,  S[b,j] = sum_l x
S comes from a 1-column ones-matmul riding the same LDWEIGHTS as the
main matmuls; SA is scaled in fp32 (a fp16 ones vector of -A/B loses
the critical 3.4e-4 and costs 2% rel err — measured).

Host pre-casts weights to fp16 (halves the dominant DMA stream):
    xt: (p, j, h, b)     x^T fp16, 8KB rows per partition
    wt: (j, p, h, i, k)  cores^T fp16, 2KB rows per partition
Granules are small at the head (fast pipeline fill) and tail (short
drain), large in the middle (amortize ACT op overhead).  Engine
streams are software-pipelined.  Core c takes out_blocks {2c, 2c+1}.
"""

import numpy as np

import concourse.bacc as bacc
import concourse.mybir as mybir
from concourse.tile import TileContext
from concourse.bass_utils import run_bass_kernel_spmd

BATCH = 128
IN_BLOCKS = 16
OUT_BLOCKS = 16
NB = 256
N_CORES = 8
I_PER_CORE = OUT_BLOCKS // N_CORES  # 2
IK = I_PER_CORE * NB  # 512

TAU, G_INF, G_MIN, L = 0.75, 2.0, 0.001, 256
B_SCALE = (G_INF - G_MIN) / (1.0 - float(np.exp(-TAU)))
A_OFF = G_MIN + B_SCALE
MULT = -(L - 1) / TAU
LN_R = -TAU / (L - 1)
_r = float(np.exp(LN_R))
DELTA = float(np.log((1 + _r) / 2) / LN_R)
C0 = 0.5 - DELTA

# poly-n constants: s(w) = w*(A2M*w + P1) + P0
_c1 = -C0 / 340.0
_g2 = -340.0 - 170.0 * _c1
C1W = -(G_MIN + B_SCALE * _c1)
A2M = 170.0 / (B_SCALE * B_SCALE)
A2B = -(170.0 * G_MIN / B_SCALE + _g2) / B_SCALE
P1 = A2B + C1W * A2M
P0 = C1W * A2B

F32 = mybir.dt.float32
F16 = mybir.dt.float16
U8 = mybir.dt.uint8

# granules: (j-list, path).  js must be contiguous and ascending.
GSPEC = [
    ([0], "ln"),
    ([1], "ln"),
    ([2, 3, 4, 5], "ln"),
    ([6, 7, 8, 9], "poly"),
    ([10, 11, 12, 13], "poly"),
    ([14], "poly"),
    ([15], "poly"),
]

_CACHE = {}


class _ForceCombinedLnExpTables:
    """Resolve Ln and Exp to the single table set containing both, so the
    ACT never reloads tables mid-kernel."""

    def __enter__(self):
        self._orig = bacc.get_activation_tables
        Ln = mybir.ActivationFunctionType.Ln
        Exp = mybir.ActivationFunctionType.Exp

        def patched(arch):
            tabs = self._orig(arch)
            out = {}
            for name, fns in tabs.items():
                if name != "natural_log_exp_and_others" and (Ln in fns or Exp in fns):
                    fns = fns - {Ln, Exp}
                out[name] = fns
            return out

        bacc.get_activation_tables = patched
        return self

    def __exit__(self, *exc):
        bacc.get_activation_tables = self._orig


def _build():
    nc = bacc.Bacc(trn_type="TRN2")
    P = 128
    NG = len(GSPEC)

    xt_d = nc.dram_tensor("xt", [P, IN_BLOCKS, 2, BATCH], F16, kind="ExternalInput")
    wt_d = nc.dram_tensor("wt", [P, IN_BLOCKS, 2, IK], F16, kind="ExternalInput")
    y_d = nc.dram_tensor("y", [BATCH, IN_BLOCKS, IK], F16, kind="ExternalOutput")

    flat = "p a b k -> p (a b k)"

    with TileContext(nc) as tc:
        with (
            tc.tile_pool(name="singles", bufs=1) as singles,
            tc.tile_pool(name="wraw", bufs=3) as wpool,
            tc.tile_pool(name="tmid", bufs=3) as tpool,
            tc.tile_pool(name="nidx", bufs=3) as npool,
            tc.tile_pool(name="texp", bufs=3) as epool,
            tc.tile_pool(name="sacc", bufs=3) as spool,
            tc.tile_pool(name="yout", bufs=3) as ypool,
            tc.tile_pool(name="yps", bufs=6, space="PSUM") as yps,
            tc.tile_pool(name="sps", bufs=1, space="PSUM") as sps,
        ):
            wt_t = [None] * NG
            t_t = [None] * NG
            n_t = [None] * NG
            e_t = [None] * NG
            sa_t = [None] * NG
            y_t = [None] * NG
            p_t = [None] * IN_BLOCKS

            def dma_w(g):
                js, _ = GSPEC[g]
                nj = len(js)
                wt_t[g] = wpool.tile([P, nj, 2, IK], F16, name="wraw", tag="wraw")
                nc.sync.dma_start(out=wt_t[g][:], in_=wt_d[:, js[0] : js[0] + nj])

            def nstage1(g):
                js, path = GSPEC[g]
                fd = len(js) * 2 * IK
                if path == "ln":
                    t_t[g] = tpool.tile([P, fd], F32, name="tln", tag="tmid")
                    nc.scalar.activation(
                        t_t[g][:],
                        wt_t[g][:].rearrange(flat),
                        mybir.ActivationFunctionType.Ln,
                        bias=bias_ln[:, 0:1],
                        scale=-1.0 / B_SCALE,
                    )
                else:
                    t_t[g] = tpool.tile([P, fd], F16, name="tgg", tag="tmid")
                    nc.vector.tensor_scalar(
                        t_t[g][:],
                        wt_t[g][:].rearrange(flat),
                        A2M,
                        P1,
                        mybir.AluOpType.mult,
                        mybir.AluOpType.add,
                    )

            def nstage2(g):
                js, path = GSPEC[g]
                fd = len(js) * 2 * IK
                n_t[g] = npool.tile([P, fd], U8, name="nidx", tag="nidx")
                if path == "ln":
                    nc.gpsimd.tensor_scalar(
                        n_t[g][:],
                        t_t[g][:],
                        MULT,
                        C0,
                        mybir.AluOpType.mult,
                        mybir.AluOpType.add,
                    )
                else:
                    hh = tpool.tile([P, fd], F16, name="thh", tag="tmid")
                    nc.vector.tensor_tensor(
                        hh[:],
                        wt_t[g][:].rearrange(flat),
                        t_t[g][:],
                        mybir.AluOpType.mult,
                    )
                    nc.vector.tensor_scalar(
                        n_t[g][:], hh[:], P0, None, mybir.AluOpType.add
                    )

            def exp_stage(g):
                js, _ = GSPEC[g]
                nj = len(js)
                e_t[g] = epool.tile([P, nj, 2, IK], F16, name="texp", tag="texp")
                nc.scalar.activation(
                    e_t[g][:].rearrange(flat),
                    n_t[g][:],
                    mybir.ActivationFunctionType.Exp,
                    bias=0.0,
                    scale=LN_R,
                )

            def mm_stage(g):
                js, _ = GSPEC[g]
                for jrel, j in enumerate(js):
                    p_t[j] = yps.tile([P, IK], F32, name="yp", tag="yp")
                    for h in range(2):
                        nc.tensor.matmul(
                            s2_ps[:, j : j + 1],
                            xt_sb[:, j, h, :],
                            ones_sb[:],
                            start=(h == 0),
                            stop=(h == 1),
                        )
                        nc.tensor.matmul(
                            p_t[j][:],
                            xt_sb[:, j, h, :],
                            e_t[g][:, jrel, h, :],
                            start=(h == 0),
                            stop=(h == 1),
                        )

            def sa_stage(g):
                js, _ = GSPEC[g]
                nj = len(js)
                sa_t[g] = spool.tile([P, nj], F32, name="sa", tag="sacc")
                nc.vector.tensor_scalar(
                    sa_t[g][:],
                    s2_ps[:, js[0] : js[0] + nj],
                    -A_OFF / B_SCALE,
                    None,
                    mybir.AluOpType.mult,
                )

            def evict_stage(g):
                js, _ = GSPEC[g]
                nj = len(js)
                y_t[g] = ypool.tile([P, nj, IK], F16, name="ysb", tag="ysb")
                for jrel, j in enumerate(js):
                    nc.vector.tensor_scalar(
                        y_t[g][:, jrel, :],
                        p_t[j][:],
                        sa_t[g][:, jrel : jrel + 1],
                        -B_SCALE,
                        mybir.AluOpType.add,
                        mybir.AluOpType.mult,
                    )

            def store_stage(g):
                js, _ = GSPEC[g]
                nc.gpsimd.dma_start(
                    out=y_d[:, js[0] : js[0] + len(js)], in_=y_t[g][:]
                )

            # --- prologue ---
            dma_w(0)
            dma_w(1)
            xt_sb = singles.tile([P, IN_BLOCKS, 2, BATCH], F16)
            nc.sync.dma_start(out=xt_sb[:], in_=xt_d[:])
            bias_ln = singles.tile([P, 1], F32)
            nc.vector.memset(bias_ln[:], A_OFF / B_SCALE)
            ones_sb = singles.tile([P, 1], F16)
            nc.vector.memset(ones_sb[:], 1.0)
            s2_ps = sps.tile([P, IN_BLOCKS], F32)

            # --- pipelined main loop ---
            nstage1(0)
            for g in range(NG):
                nstage2(g)
                if g + 2 < NG:
                    dma_w(g + 2)
                if g + 1 < NG:
                    nstage1(g + 1)
                exp_stage(g)
                mm_stage(g)
                sa_stage(g)
                evict_stage(g)
                store_stage(g)

    with _ForceCombinedLnExpTables():
        nc.compile()
    return nc


def _get_nc():
    if "nc" not in _CACHE:
        _CACHE["nc"] = _build()
    return _CACHE["nc"]


def kernel(x: np.ndarray, cores: np.ndarray, _trace=False, _trace_kwargs=None):
    x = np.asarray(x, dtype=np.float32)
    cores = np.asarray(cores, dtype=np.float32)

    xt = np.ascontiguousarray(
        x.T.reshape(IN_BLOCKS, 2, 128, BATCH)
        .transpose(2, 0, 1, 3)
        .astype(np.float16)
    )
    wt_full = (
        cores.reshape(OUT_BLOCKS, IN_BLOCKS, NB, 2, 128)  # i, j, k, h, p
        .transpose(4, 1, 3, 0, 2)  # p, j, h, i, k
        .astype(np.float16)
    )

    in_maps = []
    for c in range(N_CORES):
        wc = np.ascontiguousarray(
            wt_full[:, :, :, c * I_PER_CORE : (c + 1) * I_PER_CORE, :]
        ).reshape(128, IN_BLOCKS, 2, IK)
        in_maps.append({"xt": xt, "wt": wc})

    nc = _get_nc()
    kw = {}
    if _trace:
        kw = dict(trace=True, **(_trace_kwargs or {}))
    out = run_bass_kernel_spmd(nc, in_maps, core_ids=list(range(N_CORES)), **kw)
    if _trace:
        _CACHE["last_result"] = out
    y = np.concatenate(
        [
            r["y"]  # (b, j, (i,k))
            .astype(np.float32)
            .reshape(BATCH, IN_BLOCKS, I_PER_CORE, NB)
            .transpose(0, 2, 1, 3)
            for r in out.results
        ],
        axis=1,
    )
    return y
